# revision 30
# baseline (speedup 1.0000x reference)
"""DeepseekV2 MLA attention on 8 TRN2 NeuronCores (Bass/Tile).

Strategy (tensor-parallel over heads, 2 heads/core):
  - Host folds RMSNorm ln-weights into the following projection and fuses the
    low-rank pairs:  Wq = q_a_w @ (q_a_ln_w[:,None]*q_b_w),
                     Wk/Wv = kv_a_w[:, :512] @ (kv_a_ln_w[:,None]*kv_b_w).
    This is exact: RMSNorm(x W_a) W_b = s_t * (x W_a) (ln ⊙ W_b) with
    s_t = rsqrt(mean_k (x W_a)_k^2 + eps), a per-token scalar.
  - The per-token scalars need sum-of-squares of the (never materialized)
    low-rank activations: each core computes a 1/8 column shard of
    X@q_a_w / X@kv_a_w, squares+reduces it, and one tiny [2,2048] f32
    AllReduce shares the sums across cores.
  - RoPE pairs are permuted on host to (x1-block, x2-block) on both q and k so
    the device rotation is two contiguous row-block ops; dot products are
    invariant to the shared permutation.  The softmax scale is folded into Wq.
  - Attention runs per head in S^T[s,t] layout: scores via PE (k as stationary),
    exp on ACT (no max subtraction; |scores| ~ O(1), verified vs reference),
    causal masking by adding -1e4 to diagonal tiles pre-exp, denominator via a
    ones-vector matmul, PV accumulation with V (natural layout) stationary.
  - o_proj is row-parallel: each core emits a partial [HIDDEN, T] (transposed)
    f32 output; the host sums the 8 partials and transposes.
"""

import os
import sys

import numpy as np

for _p in ("/opt/trn_rl_repo",):
    if _p not in sys.path and os.path.isdir(_p):
        sys.path.insert(0, _p)

import ml_dtypes  # noqa: E402

BF16 = ml_dtypes.bfloat16

H = 16
D_NOPE = 128
D_ROPE = 64
D_V = 128
KV_RANK = 512
Q_RANK = 1536
HIDDEN = 2048
T = 2048
EPS = 1e-6
QK_DIM = D_NOPE + D_ROPE
SCALE = QK_DIM ** -0.5
ROPE_BASE = 10000.0

N_CORES = 8
HPC = H // N_CORES          # heads per core = 2
TCOL = 512                  # moving-operand width
NJ = T // TCOL              # 4 t-column blocks
NK = HIDDEN // 128          # 16 contraction chunks
NS = T // 128               # 16 key tiles
NEG = -1.0e4                # causal mask additive value (exp -> exact 0 in f32)

_CACHE = {}


def _build_program(debug=False):
    import concourse.bass as bass
    import concourse.mybir as mybir
    import concourse.tile as tile
    from concourse import bacc

    f32 = mybir.dt.float32
    bf16 = mybir.dt.bfloat16
    AF = mybir.ActivationFunctionType
    Alu = mybir.AluOpType

    nc = bacc.Bacc("TRN2", target_bir_lowering=False, debug=False,
                   num_devices=N_CORES)

    # ---- external I/O (per-core shards staged by the host) ----
    d_xt = nc.dram_tensor("xt", [NJ, NK, 128, TCOL], bf16, kind="ExternalInput").ap()
    d_wq = nc.dram_tensor("wq", [HIDDEN, 384], bf16, kind="ExternalInput").ap()
    d_wk = nc.dram_tensor("wk", [HIDDEN, 256], bf16, kind="ExternalInput").ap()
    d_wv = nc.dram_tensor("wv", [HIDDEN, 256], bf16, kind="ExternalInput").ap()
    d_wkpe = nc.dram_tensor("wkpe", [HIDDEN, 64], bf16, kind="ExternalInput").ap()
    d_ws1 = nc.dram_tensor("wssq1", [HIDDEN, 128], bf16, kind="ExternalInput").ap()
    d_ws2 = nc.dram_tensor("wssq2", [HIDDEN, 128], bf16, kind="ExternalInput").ap()
    d_ow = nc.dram_tensor("ow", [HPC * D_V, HIDDEN], bf16, kind="ExternalInput").ap()
    d_cos = nc.dram_tensor("cosT", [128, T], f32, kind="ExternalInput").ap()
    d_sin = nc.dram_tensor("sinT", [128, T], f32, kind="ExternalInput").ap()
    d_msw = nc.dram_tensor("mswT", [128, 128], bf16, kind="ExternalInput").ap()
    d_mask = nc.dram_tensor("maskbig", [128, 896], bf16, kind="ExternalInput").ap()
    d_out = nc.dram_tensor("out", [NK, NJ, 128, TCOL], bf16,
                           kind="ExternalOutput").ap()

    from contextlib import ExitStack

    with tile.TileContext(nc) as tc, ExitStack() as stk:
        wp = stk.enter_context(tc.tile_pool(name="weights", bufs=1))
        xt_p = stk.enter_context(tc.tile_pool(name="xtp", bufs=2))
        ap_ = stk.enter_context(tc.tile_pool(name="acts", bufs=1))
        sq_p = stk.enter_context(tc.tile_pool(name="sq", bufs=2))
        es_p = stk.enter_context(tc.tile_pool(name="es", bufs=3))
        rp = stk.enter_context(tc.tile_pool(name="rope", bufs=1))
        rd_p = stk.enter_context(tc.tile_pool(name="rdp", bufs=2))
        bc_p = stk.enter_context(tc.tile_pool(name="bcast", bufs=1))
        o_p = stk.enter_context(tc.tile_pool(name="ocopy", bufs=3))
        dram_p = stk.enter_context(tc.tile_pool(name="dram", bufs=1, space="DRAM"))
        pp = stk.enter_context(tc.tile_pool(name="pp", bufs=3, space="PSUM"))
        ps_p = stk.enter_context(tc.tile_pool(name="ps", bufs=2, space="PSUM"))
        pa_p = stk.enter_context(tc.tile_pool(name="pa", bufs=2, space="PSUM"))
        pd_p = stk.enter_context(tc.tile_pool(name="pd", bufs=1, space="PSUM"))

        # ---- resident tiles ----
        wq = wp.tile([128, NK, 384], bf16)
        wk = wp.tile([128, NK, 256], bf16)
        wv = wp.tile([128, NK, 256], bf16)
        wkpe = wp.tile([128, NK, 64], bf16)
        ws1 = wp.tile([128, NK, 128], bf16)
        ws2 = wp.tile([128, NK, 128], bf16)
        ow = wp.tile([128, HPC, HIDDEN], bf16)
        cosT = wp.tile([128, T], f32)
        sinT = wp.tile([128, T], f32)
        mswT = wp.tile([128, 128], bf16)
        maskb = wp.tile([128, 896], bf16)
        ones = wp.tile([128, 1], bf16)

        dws1 = d_ws1.rearrange("(k p) c -> p k c", p=128)
        dws2 = d_ws2.rearrange("(k p) c -> p k c", p=128)
        for g in range(2):
            gs = slice(8 * g, 8 * g + 8)
            nc.sync.dma_start(ws1[:, gs, :], dws1[:, gs, :])
            nc.sync.dma_start(ws2[:, gs, :], dws2[:, gs, :])
        dwq = d_wq.rearrange("(k p) c -> p k c", p=128)
        dwk = d_wk.rearrange("(k p) c -> p k c", p=128)
        dwv = d_wv.rearrange("(k p) c -> p k c", p=128)
        for g in range(4):
            gs = slice(4 * g, 4 * g + 4)
            nc.scalar.dma_start(wq[:, gs, :], dwq[:, gs, :])
            nc.scalar.dma_start(wk[:, gs, :], dwk[:, gs, :])
            nc.scalar.dma_start(wv[:, gs, :], dwv[:, gs, :])
        nc.scalar.dma_start(wkpe[:], d_wkpe.rearrange("(k p) c -> p k c", p=128))
        nc.scalar.dma_start(ow[:], d_ow.rearrange("(h p) c -> p h c", p=128))
        nc.scalar.dma_start(cosT[:], d_cos[:])
        nc.scalar.dma_start(sinT[:], d_sin[:])
        nc.scalar.dma_start(mswT[:], d_msw[:])
        nc.scalar.dma_start(maskb[:], d_mask[:])
        nc.gpsimd.memset(ones[:], 1.0)

        # activations (feature-major / transposed layouts)
        qn = [ap_.tile([128, T], bf16, tag=f"qn{h}", name=f"qn{h}")
              for h in range(HPC)]
        qpe = ap_.tile([128, T], bf16)          # h0 rows 0:64, h1 rows 64:128
        kn = [ap_.tile([128, T], bf16, tag=f"kn{h}", name=f"kn{h}")
              for h in range(HPC)]
        kpe_lo = ap_.tile([64, T], bf16)
        kpe = ap_.tile([128, T], bf16)          # duplicated in both 64-halves
        vna = [ap_.tile([128, NS, D_V], bf16, tag=f"v{h}", name=f"v{h}")
               for h in range(HPC)]
        att = [ap_.tile([128, T], bf16, tag=f"att{h}", name=f"att{h}")
               for h in range(HPC)]
        ssqrow_q = ap_.tile([1, T], bf16)
        ssqrow_kv = ap_.tile([1, T], bf16)
        ssqa_q = ap_.tile([1, T], bf16)
        ssqa_kv = ap_.tile([1, T], bf16)
        srow_q = ap_.tile([1, T], f32)          # rsqrt'ed scales (row layout)
        srow_kv = ap_.tile([1, T], f32)
        skvcol_raw = ap_.tile([128, NS], bf16)
        skvcol = ap_.tile([128, NS], f32)
        den_dbg = ap_.tile([1, T], f32, name="den_dbg") if debug else None
        es_dbg = ap_.tile([128, TCOL], bf16, name="es_dbg") if debug else None
        st_dbg = ap_.tile([128, TCOL], f32, name="st_dbg") if debug else None

        # ---- rope helper ----
        def rope(dst, src, rows, c):
            # dst[:rows, c] = (src*cos_dup) + Mswap @ (src*sin_dup)
            # where Mswap swaps each (x1, x2) 32-row band pair with signs.
            e = rp.tile([128, TCOL], f32, tag="re")
            f = rp.tile([128, TCOL], bf16, tag="rf")
            nc.vector.tensor_tensor(e[0:rows, :], src[0:rows, :],
                                    cosT[0:rows, c], Alu.mult)
            nc.vector.tensor_tensor(f[0:rows, :], src[0:rows, :],
                                    sinT[0:rows, c], Alu.mult)
            pr = ps_p.tile([128, TCOL], f32, tag="score")
            nc.tensor.matmul(pr[0:rows, :], mswT[0:rows, 0:rows], f[0:rows, :],
                             start=True, stop=True)
            nc.vector.tensor_tensor(dst[0:rows, c], e[0:rows, :], pr[0:rows, :],
                                    Alu.add)

        # ---- pass 1: ssq only (own xt stream) so the AllReduce fires early
        for j in range(NJ):
            c = slice(TCOL * j, TCOL * (j + 1))
            xtj = xt_p.tile([128, NK, TCOL], bf16, tag="xt", name=f"xts{j}")
            for g in range(4):
                nc.sync.dma_start(
                    xtj[:, 4 * g:4 * g + 4, :],
                    d_xt[j, 4 * g:4 * g + 4].rearrange("k p t -> p k t"))
            p0 = pp.tile([128, TCOL], f32, tag="proj")
            for k in range(NK):
                nc.tensor.matmul(p0[:], ws1[:, k, :], xtj[:, k, :],
                                 start=(k == 0), stop=(k == NK - 1))
            s0 = sq_p.tile([128, TCOL], bf16, tag="sq")
            nc.scalar.activation(s0[:], p0[:], AF.Square)
            p1 = pp.tile([128, TCOL], f32, tag="proj")
            for k in range(NK):
                nc.tensor.matmul(p1[:], ws2[:, k, :], xtj[:, k, :],
                                 start=(k == 0), stop=(k == NK - 1))
            s1 = sq_p.tile([128, TCOL], bf16, tag="sq")
            nc.scalar.activation(s1[:], p1[:], AF.Square)
            dq = pd_p.tile([1, TCOL], f32, tag="den")
            nc.tensor.matmul(dq[:], ones[:, :], s0[:], start=True, stop=False)
            nc.tensor.matmul(dq[:], ones[0:64, :], s1[0:64, :],
                             start=False, stop=True)
            nc.vector.tensor_copy(ssqrow_q[0:1, c], dq[:])
            dk = pd_p.tile([1, TCOL], f32, tag="den")
            nc.tensor.matmul(dk[:], ones[64:128, :], s1[64:128, :],
                             start=True, stop=True)
            nc.vector.tensor_copy(ssqrow_kv[0:1, c], dk[:])

        # ---- AllReduce of the ssq rows (overlaps with the projection pass)
        cc_in = dram_p.tile([2, T], bf16)
        cc_out = dram_p.tile([2, T], bf16)
        nc.gpsimd.dma_start(cc_in[0:1, :], ssqrow_q[:])
        nc.gpsimd.dma_start(cc_in[1:2, :], ssqrow_kv[:])
        nc.gpsimd.collective_compute(
            "AllReduce", Alu.add,
            replica_groups=[list(range(N_CORES))],
            ins=[cc_in.opt()], outs=[cc_out.opt()],
        )
        nc.gpsimd.dma_start(ssqa_q[:], cc_out[0:1, :])
        nc.gpsimd.dma_start(ssqa_kv[:], cc_out[1:2, :])
        nc.gpsimd.dma_start(skvcol_raw[:],
                            cc_out[1:2, :].rearrange("o (b p) -> (o p) b", p=128))

        # ---- pass 2: fused projections + V (re-stream xt) ----
        for j in range(NJ):
            c = slice(TCOL * j, TCOL * (j + 1))
            xtj = xt_p.tile([128, NK, TCOL], bf16, tag="xt", name=f"xt{j}")
            for g in range(4):
                nc.sync.dma_start(
                    xtj[:, 4 * g:4 * g + 4, :],
                    d_xt[j, 4 * g:4 * g + 4].rearrange("k p t -> p k t"))
            for h in range(HPC):
                p = pp.tile([128, TCOL], f32, tag="proj")
                for k in range(NK):
                    nc.tensor.matmul(p[:], wq[:, k, 128 * h:128 * h + 128],
                                     xtj[:, k, :],
                                     start=(k == 0), stop=(k == NK - 1))
                nc.vector.tensor_copy(qn[h][:, c], p[:])
            p = pp.tile([128, TCOL], f32, tag="proj")
            for k in range(NK):
                nc.tensor.matmul(p[:], wq[:, k, 256:384], xtj[:, k, :],
                                 start=(k == 0), stop=(k == NK - 1))
            rope(qpe, p, 128, c)
            for h in range(HPC):
                p = pp.tile([128, TCOL], f32, tag="proj")
                for k in range(NK):
                    nc.tensor.matmul(p[:], wk[:, k, 128 * h:128 * h + 128],
                                     xtj[:, k, :],
                                     start=(k == 0), stop=(k == NK - 1))
                nc.vector.tensor_copy(kn[h][:, c], p[:])
            p = pp.tile([128, TCOL], f32, tag="proj")
            for k in range(NK):
                nc.tensor.matmul(p[0:64, :], wkpe[:, k, :], xtj[:, k, :],
                                 start=(k == 0), stop=(k == NK - 1))
            rope(kpe_lo, p, 64, c)

            # V (natural [s, dv]) for both heads: X^T chunk stationary
            for sl in range(4):
                si = 4 * j + sl
                pv = pp.tile([128, TCOL], f32, tag="proj")
                for k in range(NK):
                    nc.tensor.matmul(pv[:, 0:256],
                                     xtj[:, k, 128 * sl:128 * sl + 128],
                                     wv[:, k, :], start=(k == 0),
                                     stop=(k == NK - 1))
                for h in range(HPC):
                    nc.vector.tensor_copy(vna[h][:, si, :],
                                          pv[:, 128 * h:128 * h + 128])

        # duplicate k_pe into both 64-row halves for per-head base alignment
        nc.sync.dma_start(kpe[0:64, :], kpe_lo[:])
        nc.sync.dma_start(kpe[64:128, :], kpe_lo[:])

        # scales: s = rsqrt(ssq/rank + eps)
        for ssrc, sdst, rank in ((ssqa_q, srow_q, Q_RANK),
                                 (ssqa_kv, srow_kv, KV_RANK)):
            nc.vector.tensor_scalar(sdst[:], ssrc[:],
                                    1.0 / rank, EPS, Alu.mult, Alu.add)
            nc.vector.reciprocal(sdst[:], sdst[:])
            nc.scalar.activation(sdst[:], sdst[:], AF.Sqrt)
        nc.vector.tensor_scalar(skvcol[:], skvcol_raw[:],
                                1.0 / KV_RANK, EPS, Alu.mult, Alu.add)
        nc.vector.reciprocal(skvcol[:], skvcol[:])
        nc.scalar.activation(skvcol[:], skvcol[:], AF.Sqrt)

        # ---- per-column: late scaling then attention for both heads ----
        # (scaling depends on the AllReduce; attention (h, j) only needs
        #  columns <= j scaled, so interleaving maximizes PE overlap)
        for j in range(NJ):
            c = slice(TCOL * j, TCOL * (j + 1))
            sqB = bc_p.tile([128, TCOL], f32, tag="sqB")
            nc.gpsimd.partition_broadcast(sqB[:], srow_q[0:1, c])
            skvB = bc_p.tile([128, TCOL], f32, tag="skvB")
            nc.gpsimd.partition_broadcast(skvB[:], srow_kv[0:1, c])
            for h in range(HPC):
                nc.vector.tensor_tensor(qn[h][:, c], qn[h][:, c], sqB[:],
                                        Alu.mult)
                nc.vector.tensor_tensor(kn[h][:, c], kn[h][:, c], skvB[:],
                                        Alu.mult)
            nc.vector.tensor_tensor(qpe[:, c], qpe[:, c], sqB[:], Alu.mult)
            for sl in range(4):
                si = 4 * j + sl
                for h in range(HPC):
                    nc.vector.tensor_scalar_mul(vna[h][:, si, :],
                                                vna[h][:, si, :],
                                                skvcol[:, si:si + 1])

            # attention in S^T[s, t] layout, causal block-skip
            for h in range(HPC):
                pa = pa_p.tile([128, TCOL], f32, tag="attn")
                pden = pd_p.tile([1, TCOL], f32, tag="den")
                n_s = 4 * (j + 1)
                for i in range(n_s):
                    st = ps_p.tile([128, TCOL], f32, tag="score")
                    nc.tensor.matmul(st[:], kn[h][:, 128 * i:128 * i + 128],
                                     qn[h][:, c], start=True, stop=False)
                    nc.tensor.matmul(st[:],
                                     kpe[64 * h:64 * h + 64,
                                         128 * i:128 * i + 128],
                                     qpe[64 * h:64 * h + 64, c],
                                     start=False, stop=True)
                    if i >= 4 * j:
                        ko = i - 4 * j
                        nc.vector.tensor_tensor(
                            st[:], st[:],
                            maskb[:, 384 - 128 * ko:896 - 128 * ko], Alu.add)
                    es = es_p.tile([128, TCOL], bf16, tag="es")
                    if debug and h == 0 and j == 0 and i == 0:
                        nc.vector.tensor_copy(st_dbg[:], st[:])
                    nc.scalar.activation(es[:], st[:], AF.Exp)
                    if debug and h == 0 and j == 0 and i == 0:
                        nc.vector.tensor_copy(es_dbg[:], es[:])
                    nc.tensor.matmul(pden[:], ones[:, :], es[:],
                                     start=(i == 0), stop=(i == n_s - 1),
                                     skip_group_check=True)
                    nc.tensor.matmul(pa[:], vna[h][:, i, :], es[:],
                                     start=(i == 0), stop=(i == n_s - 1),
                                     skip_group_check=True)
                if debug and h == 0:
                    nc.vector.tensor_copy(den_dbg[0:1, c], pden[:])
                rden = rd_p.tile([1, TCOL], f32, tag="rden")
                nc.vector.reciprocal(rden[:], pden[:])
                rdB = rd_p.tile([128, TCOL], f32, tag="rdB")
                nc.gpsimd.partition_broadcast(rdB[:], rden[:])
                nc.vector.tensor_tensor(att[h][:, c], pa[:], rdB[:], Alu.mult)

        # ---- phase 4: o_proj (row-parallel partial, transposed output) ----
        for m in range(NK):
            ot = o_p.tile([128, NJ, TCOL], bf16, tag="ot")
            for j in range(NJ):
                c = slice(TCOL * j, TCOL * (j + 1))
                po = pp.tile([128, TCOL], f32, tag="proj")
                for h in range(HPC):
                    nc.tensor.matmul(po[:], ow[:, h, 128 * m:128 * m + 128],
                                     att[h][:, c],
                                     start=(h == 0), stop=(h == HPC - 1))
                nc.vector.tensor_copy(ot[:, j, :], po[:])
            dom = d_out[m].rearrange("j p t -> p j t")
            nc.sync.dma_start(dom[:, 0:2, :], ot[:, 0:2, :])
            nc.sync.dma_start(dom[:, 2:4, :], ot[:, 2:4, :])

        if debug:
            dbg_specs = [
                ("dbg_ssqrow_q", ssqrow_q, [1, T], bf16),
                ("dbg_ssqa_q", ssqa_q, [1, T], bf16),
                ("dbg_srow_q", srow_q, [1, T], f32),
                ("dbg_srow_kv", srow_kv, [1, T], f32),
                ("dbg_skvcol", skvcol, [128, NS], f32),
                ("dbg_qn0", qn[0], [128, T], bf16),
                ("dbg_qpe", qpe, [128, T], bf16),
                ("dbg_kn0", kn[0], [128, T], bf16),
                ("dbg_kpe", kpe, [128, T], bf16),
                ("dbg_v0", vna[0], [128, NS, D_V], bf16),
                ("dbg_att0", att[0], [128, T], bf16),
                ("dbg_den", den_dbg, [1, T], f32),
                ("dbg_es00", es_dbg, [128, TCOL], bf16),
                ("dbg_st00", st_dbg, [128, TCOL], f32),
            ]
            for nm, src_t, shp, dt in dbg_specs:
                dd = nc.dram_tensor(nm, shp, dt, kind="ExternalOutput").ap()
                nc.sync.dma_start(dd[:], src_t[:])

    nc.compile()
    return nc


def _host_prep(positions, hidden_states, q_a_w, q_a_ln_w, q_b_w,
               kv_a_w, kv_a_ln_w, kv_b_w, o_w):
    pos = np.asarray(positions, dtype=np.float32)
    hs = np.asarray(hidden_states, dtype=np.float32)
    q_a_w = np.asarray(q_a_w, dtype=np.float32)
    q_b_w = np.asarray(q_b_w, dtype=np.float32) * np.asarray(
        q_a_ln_w, dtype=np.float32)[:, None]
    kv_a_w = np.asarray(kv_a_w, dtype=np.float32)
    kv_b_w = np.asarray(kv_b_w, dtype=np.float32) * np.asarray(
        kv_a_ln_w, dtype=np.float32)[:, None]
    o_w = np.asarray(o_w, dtype=np.float32)

    # fused weights
    wq_full = (q_a_w @ q_b_w).reshape(HIDDEN, H, QK_DIM) * SCALE
    kvb = kv_b_w.reshape(KV_RANK, H, D_NOPE + D_V)
    wk_full = kv_a_w[:, :KV_RANK] @ kvb[:, :, :D_NOPE].reshape(KV_RANK, -1)
    wk_full = wk_full.reshape(HIDDEN, H, D_NOPE)
    wv_full = kv_a_w[:, :KV_RANK] @ kvb[:, :, D_NOPE:].reshape(KV_RANK, -1)
    wv_full = wv_full.reshape(HIDDEN, H, D_V)

    # rope pair permutation: interleaved (0::2, 1::2) -> (x1 block | x2 block)
    qpe_cols = wq_full[:, :, D_NOPE:]
    qpe_perm = np.concatenate([qpe_cols[:, :, 0::2], qpe_cols[:, :, 1::2]],
                              axis=2)  # [HIDDEN, H, 64]
    wkpe = kv_a_w[:, KV_RANK:]
    wkpe_perm = np.concatenate([wkpe[:, 0::2], wkpe[:, 1::2]], axis=1)

    inv_freq = 1.0 / (ROPE_BASE ** (np.arange(0, D_ROPE, 2,
                                              dtype=np.float32) / D_ROPE))
    freqs = pos[None, :] * inv_freq[:, None]           # [32, T]
    cosT = np.tile(np.cos(freqs).astype(np.float32), (4, 1))   # [128, T]
    sinT = np.tile(np.sin(freqs).astype(np.float32), (4, 1))

    # band-swap-with-sign matrix: o = e + Msw @ f  (per 64-row block:
    # rows 0:32 get -f[32:64], rows 32:64 get +f[0:32])
    msw = np.zeros((128, 128), dtype=np.float32)
    for q in range(2):
        for i in range(32):
            msw[64 * q + i, 64 * q + 32 + i] = -1.0
            msw[64 * q + 32 + i, 64 * q + i] = 1.0
    mswT = np.ascontiguousarray(msw.T).astype(BF16)

    # big causal mask: maskb[s, col] = 0 if col >= s + 384 else NEG
    col = np.arange(896)[None, :]
    s_ = np.arange(128)[:, None]
    maskb = np.where(col >= s_ + 384, 0.0, NEG).astype(BF16)

    xt = np.ascontiguousarray(
        hs.T.reshape(NK, 128, NJ, TCOL).transpose(2, 0, 1, 3)).astype(BF16)

    in_maps = []
    for cidx in range(N_CORES):
        h0 = HPC * cidx
        wq_c = np.concatenate(
            [wq_full[:, h0 + h, :D_NOPE] for h in range(HPC)]
            + [qpe_perm[:, h0 + h, :] for h in range(HPC)], axis=1)
        wk_c = np.concatenate(
            [wk_full[:, h0 + h, :] for h in range(HPC)], axis=1)
        wv_c = np.concatenate(
            [wv_full[:, h0 + h, :] for h in range(HPC)], axis=1)
        ws1 = q_a_w[:, 192 * cidx:192 * cidx + 128]
        ws2 = np.concatenate(
            [q_a_w[:, 192 * cidx + 128:192 * (cidx + 1)],
             kv_a_w[:, 64 * cidx:64 * (cidx + 1)]], axis=1)
        ow_c = o_w[D_V * h0:D_V * (h0 + HPC), :]
        in_maps.append({
            "xt": xt,
            "wq": np.ascontiguousarray(wq_c).astype(BF16),
            "wk": np.ascontiguousarray(wk_c).astype(BF16),
            "wv": np.ascontiguousarray(wv_c).astype(BF16),
            "wkpe": np.ascontiguousarray(wkpe_perm).astype(BF16),
            "wssq1": np.ascontiguousarray(ws1).astype(BF16),
            "wssq2": np.ascontiguousarray(ws2).astype(BF16),
            "ow": np.ascontiguousarray(ow_c).astype(BF16),
            "cosT": cosT,
            "sinT": sinT,
            "mswT": mswT,
            "maskbig": maskb,
        })
    return in_maps


def kernel(**inputs):
    from concourse.bass_utils import run_bass_kernel_spmd

    dbg = bool(int(os.environ.get("BASSK_DEBUG", "0")))
    key = "nc_dbg" if dbg else "nc"
    if key not in _CACHE:
        _CACHE[key] = _build_program(debug=dbg)
    nc = _CACHE[key]

    in_maps = _host_prep(**inputs)
    trace = bool(int(os.environ.get("BASSK_TRACE", "0")))
    tmpdir = os.environ.get("BASSK_TMPDIR") or None
    if tmpdir:
        os.makedirs(tmpdir, exist_ok=True)
    res = run_bass_kernel_spmd(nc, in_maps, core_ids=list(range(N_CORES)),
                               trace=trace, tmpdir=tmpdir)
    _CACHE["last_exec_time_ns"] = res.exec_time_ns
    _CACHE["last_results"] = res.results
    outT = np.zeros((NK, NJ, 128, TCOL), dtype=np.float32)
    for r in res.results:
        outT += np.asarray(r["out"], dtype=np.float32)
    outT = outT.transpose(0, 2, 1, 3).reshape(HIDDEN, T)
    return np.ascontiguousarray(outT.T)


# revision 32
# speedup vs baseline: 1.1804x; 1.1804x over previous
"""DeepseekV2 MLA attention on 8 TRN2 NeuronCores (Bass/Tile).

Strategy (tensor-parallel over heads, 2 heads/core):
  - Host folds RMSNorm ln-weights into the following projection and fuses the
    low-rank pairs:  Wq = q_a_w @ (q_a_ln_w[:,None]*q_b_w),
                     Wk/Wv = kv_a_w[:, :512] @ (kv_a_ln_w[:,None]*kv_b_w).
    This is exact: RMSNorm(x W_a) W_b = s_t * (x W_a) (ln ⊙ W_b) with
    s_t = rsqrt(mean_k (x W_a)_k^2 + eps), a per-token scalar.
  - The per-token scalars need sum-of-squares of the (never materialized)
    low-rank activations: each core computes a 1/8 column shard of
    X@q_a_w / X@kv_a_w, squares+reduces it, and one tiny [2,2048] f32
    AllReduce shares the sums across cores.
  - RoPE pairs are permuted on host to (x1-block, x2-block) on both q and k so
    the device rotation is two contiguous row-block ops; dot products are
    invariant to the shared permutation.  The softmax scale is folded into Wq.
  - Attention runs per head in S^T[s,t] layout: scores via PE (k as stationary),
    exp on ACT (no max subtraction; |scores| ~ O(1), verified vs reference),
    causal masking by adding -1e4 to diagonal tiles pre-exp, denominator via a
    ones-vector matmul, PV accumulation with V (natural layout) stationary.
  - o_proj is row-parallel: each core emits a partial [HIDDEN, T] (transposed)
    f32 output; the host sums the 8 partials and transposes.
"""

import os
import sys

import numpy as np

for _p in ("/opt/trn_rl_repo",):
    if _p not in sys.path and os.path.isdir(_p):
        sys.path.insert(0, _p)

import ml_dtypes  # noqa: E402

BF16 = ml_dtypes.bfloat16

H = 16
D_NOPE = 128
D_ROPE = 64
D_V = 128
KV_RANK = 512
Q_RANK = 1536
HIDDEN = 2048
T = 2048
EPS = 1e-6
QK_DIM = D_NOPE + D_ROPE
SCALE = QK_DIM ** -0.5
ROPE_BASE = 10000.0

N_CORES = 8
HPC = H // N_CORES          # heads per core = 2
TCOL = 512                  # moving-operand width
NJ = T // TCOL              # 4 t-column blocks
NK = HIDDEN // 128          # 16 contraction chunks
NS = T // 128               # 16 key tiles
NEG = -1.0e4                # causal mask additive value (exp -> exact 0 in f32)

_CACHE = {}


def _build_program(debug=False):
    import concourse.bass as bass
    import concourse.mybir as mybir
    import concourse.tile as tile
    from concourse import bacc

    f32 = mybir.dt.float32
    bf16 = mybir.dt.bfloat16
    AF = mybir.ActivationFunctionType
    Alu = mybir.AluOpType

    nc = bacc.Bacc("TRN2", target_bir_lowering=False, debug=False,
                   num_devices=N_CORES)

    # ---- external I/O (per-core shards staged by the host) ----
    d_xt = nc.dram_tensor("xt", [NJ, NK, 128, TCOL], bf16, kind="ExternalInput").ap()
    d_wq = nc.dram_tensor("wq", [HIDDEN, 384], bf16, kind="ExternalInput").ap()
    d_wk = nc.dram_tensor("wk", [HIDDEN, 256], bf16, kind="ExternalInput").ap()
    d_wv = nc.dram_tensor("wv", [HIDDEN, 256], bf16, kind="ExternalInput").ap()
    d_wkpe = nc.dram_tensor("wkpe", [HIDDEN, 64], bf16, kind="ExternalInput").ap()
    d_ws1 = nc.dram_tensor("wssq1", [HIDDEN, 128], bf16, kind="ExternalInput").ap()
    d_ws2 = nc.dram_tensor("wssq2", [HIDDEN, 128], bf16, kind="ExternalInput").ap()
    d_ow = nc.dram_tensor("ow", [HPC * D_V, HIDDEN], bf16, kind="ExternalInput").ap()
    d_cos = nc.dram_tensor("cosT", [128, T], f32, kind="ExternalInput").ap()
    d_sin = nc.dram_tensor("sinT", [128, T], f32, kind="ExternalInput").ap()
    d_msw = nc.dram_tensor("mswT", [128, 128], bf16, kind="ExternalInput").ap()
    d_mask = nc.dram_tensor("maskbig", [128, 896], bf16, kind="ExternalInput").ap()
    d_out = nc.dram_tensor("out", [NK, NJ, 128, TCOL], bf16,
                           kind="ExternalOutput").ap()

    from contextlib import ExitStack

    with tile.TileContext(nc) as tc, ExitStack() as stk:
        wp = stk.enter_context(tc.tile_pool(name="weights", bufs=1))
        xt_p = stk.enter_context(tc.tile_pool(name="xtp", bufs=2))
        ap_ = stk.enter_context(tc.tile_pool(name="acts", bufs=1))
        sq_p = stk.enter_context(tc.tile_pool(name="sq", bufs=2))
        es_p = stk.enter_context(tc.tile_pool(name="es", bufs=3))
        rp = stk.enter_context(tc.tile_pool(name="rope", bufs=1))
        rd_p = stk.enter_context(tc.tile_pool(name="rdp", bufs=2))
        bc_p = stk.enter_context(tc.tile_pool(name="bcast", bufs=1))
        o_p = stk.enter_context(tc.tile_pool(name="ocopy", bufs=3))
        dram_p = stk.enter_context(tc.tile_pool(name="dram", bufs=1, space="DRAM"))
        pp = stk.enter_context(tc.tile_pool(name="pp", bufs=3, space="PSUM"))
        ps_p = stk.enter_context(tc.tile_pool(name="ps", bufs=2, space="PSUM"))
        pa_p = stk.enter_context(tc.tile_pool(name="pa", bufs=2, space="PSUM"))
        pd_p = stk.enter_context(tc.tile_pool(name="pd", bufs=1, space="PSUM"))

        # ---- resident tiles ----
        wq = wp.tile([128, NK, 384], bf16)
        wk = wp.tile([128, NK, 256], bf16)
        wv = wp.tile([128, NK, 256], bf16)
        wkpe = wp.tile([128, NK, 64], bf16)
        ws1 = wp.tile([128, NK, 128], bf16)
        ws2 = wp.tile([128, NK, 128], bf16)
        ow = wp.tile([128, HPC, HIDDEN], bf16)
        cosT = wp.tile([128, T], f32)
        sinT = wp.tile([128, T], f32)
        mswT = wp.tile([128, 128], bf16)
        maskb = wp.tile([128, 896], bf16)
        ones = wp.tile([128, 1], bf16)

        dws1 = d_ws1.rearrange("(k p) c -> p k c", p=128)
        dws2 = d_ws2.rearrange("(k p) c -> p k c", p=128)
        for g in range(2):
            gs = slice(8 * g, 8 * g + 8)
            nc.sync.dma_start(ws1[:, gs, :], dws1[:, gs, :])
            nc.sync.dma_start(ws2[:, gs, :], dws2[:, gs, :])
        nc.gpsimd.memset(ones[:], 1.0)

        # activations (feature-major / transposed layouts)
        qn = [ap_.tile([128, T], bf16, tag=f"qn{h}", name=f"qn{h}")
              for h in range(HPC)]
        qpe = ap_.tile([128, T], bf16)          # h0 rows 0:64, h1 rows 64:128
        kn = [ap_.tile([128, T], bf16, tag=f"kn{h}", name=f"kn{h}")
              for h in range(HPC)]
        kpe_lo = ap_.tile([64, T], bf16)
        kpe = ap_.tile([128, T], bf16)          # duplicated in both 64-halves
        vna = [ap_.tile([128, NS, D_V], bf16, tag=f"v{h}", name=f"v{h}")
               for h in range(HPC)]
        att = [ap_.tile([128, T], bf16, tag=f"att{h}", name=f"att{h}")
               for h in range(HPC)]
        ssqrow_q = ap_.tile([1, T], bf16)
        ssqrow_kv = ap_.tile([1, T], bf16)
        ssqa_q = ap_.tile([1, T], bf16)
        ssqa_kv = ap_.tile([1, T], bf16)
        srow_q = ap_.tile([1, T], f32)          # rsqrt'ed scales (row layout)
        srow_kv = ap_.tile([1, T], f32)
        skvcol_raw = ap_.tile([128, NS], bf16)
        skvcol = ap_.tile([128, NS], f32)
        den_dbg = ap_.tile([1, T], f32, name="den_dbg") if debug else None
        es_dbg = ap_.tile([128, TCOL], bf16, name="es_dbg") if debug else None
        st_dbg = ap_.tile([128, TCOL], f32, name="st_dbg") if debug else None

        # ---- rope helper ----
        def rope(dst, src, rows, c):
            # dst[:rows, c] = (src*cos_dup) + Mswap @ (src*sin_dup)
            # where Mswap swaps each (x1, x2) 32-row band pair with signs.
            e = rp.tile([128, TCOL], f32, tag="re")
            f = rp.tile([128, TCOL], bf16, tag="rf")
            nc.vector.tensor_tensor(e[0:rows, :], src[0:rows, :],
                                    cosT[0:rows, c], Alu.mult)
            nc.vector.tensor_tensor(f[0:rows, :], src[0:rows, :],
                                    sinT[0:rows, c], Alu.mult)
            pr = ps_p.tile([128, TCOL], f32, tag="score")
            nc.tensor.matmul(pr[0:rows, :], mswT[0:rows, 0:rows], f[0:rows, :],
                             start=True, stop=True)
            nc.vector.tensor_tensor(dst[0:rows, c], e[0:rows, :], pr[0:rows, :],
                                    Alu.add)

        # ---- pass 1: ssq only (own xt stream) so the AllReduce fires early
        for j in range(NJ):
            c = slice(TCOL * j, TCOL * (j + 1))
            xtj = xt_p.tile([128, NK, TCOL], bf16, tag="xt", name=f"xts{j}")
            for g in range(8):
                nc.sync.dma_start(
                    xtj[:, 2 * g:2 * g + 2, :],
                    d_xt[j, 2 * g:2 * g + 2].rearrange("k p t -> p k t"))
            p0 = pp.tile([128, TCOL], f32, tag="proj")
            for k in range(NK):
                nc.tensor.matmul(p0[:], ws1[:, k, :], xtj[:, k, :],
                                 start=(k == 0), stop=(k == NK - 1))
            s0 = sq_p.tile([128, TCOL], bf16, tag="sq")
            nc.scalar.activation(s0[:], p0[:], AF.Square)
            p1 = pp.tile([128, TCOL], f32, tag="proj")
            for k in range(NK):
                nc.tensor.matmul(p1[:], ws2[:, k, :], xtj[:, k, :],
                                 start=(k == 0), stop=(k == NK - 1))
            s1 = sq_p.tile([128, TCOL], bf16, tag="sq")
            nc.scalar.activation(s1[:], p1[:], AF.Square)
            dq = pd_p.tile([1, TCOL], f32, tag="den")
            nc.tensor.matmul(dq[:], ones[:, :], s0[:], start=True, stop=False)
            nc.tensor.matmul(dq[:], ones[0:64, :], s1[0:64, :],
                             start=False, stop=True)
            nc.vector.tensor_copy(ssqrow_q[0:1, c], dq[:])
            dk = pd_p.tile([1, TCOL], f32, tag="den")
            nc.tensor.matmul(dk[:], ones[64:128, :], s1[64:128, :],
                             start=True, stop=True)
            nc.vector.tensor_copy(ssqrow_kv[0:1, c], dk[:])

        # remaining weights: issued after pass-1 xt so ssq isn't starved
        nc.sync.dma_start(wq[:], d_wq.rearrange("(k p) c -> p k c", p=128))
        nc.sync.dma_start(wk[:], d_wk.rearrange("(k p) c -> p k c", p=128))
        nc.sync.dma_start(wv[:], d_wv.rearrange("(k p) c -> p k c", p=128))
        nc.sync.dma_start(wkpe[:], d_wkpe.rearrange("(k p) c -> p k c", p=128))
        nc.sync.dma_start(ow[:], d_ow.rearrange("(h p) c -> p h c", p=128))
        nc.sync.dma_start(cosT[:], d_cos[:])
        nc.sync.dma_start(sinT[:], d_sin[:])
        nc.sync.dma_start(mswT[:], d_msw[:])
        nc.sync.dma_start(maskb[:], d_mask[:])

        # ---- AllReduce of the ssq rows (overlaps with the projection pass)
        cc_in = dram_p.tile([2, T], bf16)
        cc_out = dram_p.tile([2, T], bf16)
        nc.gpsimd.dma_start(cc_in[0:1, :], ssqrow_q[:])
        nc.gpsimd.dma_start(cc_in[1:2, :], ssqrow_kv[:])
        nc.gpsimd.collective_compute(
            "AllReduce", Alu.add,
            replica_groups=[list(range(N_CORES))],
            ins=[cc_in.opt()], outs=[cc_out.opt()],
        )
        nc.gpsimd.dma_start(ssqa_q[:], cc_out[0:1, :])
        nc.gpsimd.dma_start(ssqa_kv[:], cc_out[1:2, :])
        nc.gpsimd.dma_start(skvcol_raw[:],
                            cc_out[1:2, :].rearrange("o (b p) -> (o p) b", p=128))

        # ---- pass 2: fused projections + V (re-stream xt) ----
        for j in range(NJ):
            c = slice(TCOL * j, TCOL * (j + 1))
            xtj = xt_p.tile([128, NK, TCOL], bf16, tag="xt", name=f"xt{j}")
            for g in range(4):
                nc.sync.dma_start(
                    xtj[:, 4 * g:4 * g + 4, :],
                    d_xt[j, 4 * g:4 * g + 4].rearrange("k p t -> p k t"))
            for h in range(HPC):
                p = pp.tile([128, TCOL], f32, tag="proj")
                for k in range(NK):
                    nc.tensor.matmul(p[:], wq[:, k, 128 * h:128 * h + 128],
                                     xtj[:, k, :],
                                     start=(k == 0), stop=(k == NK - 1))
                nc.vector.tensor_copy(qn[h][:, c], p[:])
            p = pp.tile([128, TCOL], f32, tag="proj")
            for k in range(NK):
                nc.tensor.matmul(p[:], wq[:, k, 256:384], xtj[:, k, :],
                                 start=(k == 0), stop=(k == NK - 1))
            rope(qpe, p, 128, c)
            for h in range(HPC):
                p = pp.tile([128, TCOL], f32, tag="proj")
                for k in range(NK):
                    nc.tensor.matmul(p[:], wk[:, k, 128 * h:128 * h + 128],
                                     xtj[:, k, :],
                                     start=(k == 0), stop=(k == NK - 1))
                nc.vector.tensor_copy(kn[h][:, c], p[:])
            p = pp.tile([128, TCOL], f32, tag="proj")
            for k in range(NK):
                nc.tensor.matmul(p[0:64, :], wkpe[:, k, :], xtj[:, k, :],
                                 start=(k == 0), stop=(k == NK - 1))
            rope(kpe_lo, p, 64, c)

            # V (natural [s, dv]) for both heads: X^T chunk stationary
            for sl in range(4):
                si = 4 * j + sl
                pv = pp.tile([128, TCOL], f32, tag="proj")
                for k in range(NK):
                    nc.tensor.matmul(pv[:, 0:256],
                                     xtj[:, k, 128 * sl:128 * sl + 128],
                                     wv[:, k, :], start=(k == 0),
                                     stop=(k == NK - 1))
                for h in range(HPC):
                    nc.vector.tensor_copy(vna[h][:, si, :],
                                          pv[:, 128 * h:128 * h + 128])

        # duplicate k_pe into both 64-row halves for per-head base alignment
        nc.sync.dma_start(kpe[0:64, :], kpe_lo[:])
        nc.sync.dma_start(kpe[64:128, :], kpe_lo[:])

        # scales: s = rsqrt(ssq/rank + eps)
        for ssrc, sdst, rank in ((ssqa_q, srow_q, Q_RANK),
                                 (ssqa_kv, srow_kv, KV_RANK)):
            nc.vector.tensor_scalar(sdst[:], ssrc[:],
                                    1.0 / rank, EPS, Alu.mult, Alu.add)
            nc.vector.reciprocal(sdst[:], sdst[:])
            nc.scalar.activation(sdst[:], sdst[:], AF.Sqrt)
        nc.vector.tensor_scalar(skvcol[:], skvcol_raw[:],
                                1.0 / KV_RANK, EPS, Alu.mult, Alu.add)
        nc.vector.reciprocal(skvcol[:], skvcol[:])
        nc.scalar.activation(skvcol[:], skvcol[:], AF.Sqrt)

        # ---- per-column: late scaling then attention for both heads ----
        # (scaling depends on the AllReduce; attention (h, j) only needs
        #  columns <= j scaled, so interleaving maximizes PE overlap)
        for j in range(NJ):
            c = slice(TCOL * j, TCOL * (j + 1))
            sqB = bc_p.tile([128, TCOL], f32, tag="sqB")
            nc.gpsimd.partition_broadcast(sqB[:], srow_q[0:1, c])
            skvB = bc_p.tile([128, TCOL], f32, tag="skvB")
            nc.gpsimd.partition_broadcast(skvB[:], srow_kv[0:1, c])
            for h in range(HPC):
                nc.vector.tensor_tensor(qn[h][:, c], qn[h][:, c], sqB[:],
                                        Alu.mult)
                nc.vector.tensor_tensor(kn[h][:, c], kn[h][:, c], skvB[:],
                                        Alu.mult)
            nc.vector.tensor_tensor(qpe[:, c], qpe[:, c], sqB[:], Alu.mult)
            for sl in range(4):
                si = 4 * j + sl
                for h in range(HPC):
                    nc.vector.tensor_scalar_mul(vna[h][:, si, :],
                                                vna[h][:, si, :],
                                                skvcol[:, si:si + 1])

            # attention in S^T[s, t] layout, causal block-skip
            for h in range(HPC):
                pa = pa_p.tile([128, TCOL], f32, tag="attn")
                pden = pd_p.tile([1, TCOL], f32, tag="den")
                n_s = 4 * (j + 1)
                for i in range(n_s):
                    st = ps_p.tile([128, TCOL], f32, tag="score")
                    nc.tensor.matmul(st[:], kn[h][:, 128 * i:128 * i + 128],
                                     qn[h][:, c], start=True, stop=False)
                    nc.tensor.matmul(st[:],
                                     kpe[64 * h:64 * h + 64,
                                         128 * i:128 * i + 128],
                                     qpe[64 * h:64 * h + 64, c],
                                     start=False, stop=True)
                    if i >= 4 * j:
                        ko = i - 4 * j
                        nc.vector.tensor_tensor(
                            st[:], st[:],
                            maskb[:, 384 - 128 * ko:896 - 128 * ko], Alu.add)
                    es = es_p.tile([128, TCOL], bf16, tag="es")
                    if debug and h == 0 and j == 0 and i == 0:
                        nc.vector.tensor_copy(st_dbg[:], st[:])
                    nc.scalar.activation(es[:], st[:], AF.Exp)
                    if debug and h == 0 and j == 0 and i == 0:
                        nc.vector.tensor_copy(es_dbg[:], es[:])
                    nc.tensor.matmul(pden[:], ones[:, :], es[:],
                                     start=(i == 0), stop=(i == n_s - 1),
                                     skip_group_check=True)
                    nc.tensor.matmul(pa[:], vna[h][:, i, :], es[:],
                                     start=(i == 0), stop=(i == n_s - 1),
                                     skip_group_check=True)
                if debug and h == 0:
                    nc.vector.tensor_copy(den_dbg[0:1, c], pden[:])
                rden = rd_p.tile([1, TCOL], f32, tag="rden")
                nc.vector.reciprocal(rden[:], pden[:])
                rdB = rd_p.tile([128, TCOL], f32, tag="rdB")
                nc.gpsimd.partition_broadcast(rdB[:], rden[:])
                nc.vector.tensor_tensor(att[h][:, c], pa[:], rdB[:], Alu.mult)

        # ---- phase 4: o_proj (row-parallel partial, transposed output) ----
        for m in range(NK):
            ot = o_p.tile([128, NJ, TCOL], bf16, tag="ot")
            for j in range(NJ):
                c = slice(TCOL * j, TCOL * (j + 1))
                po = pp.tile([128, TCOL], f32, tag="proj")
                for h in range(HPC):
                    nc.tensor.matmul(po[:], ow[:, h, 128 * m:128 * m + 128],
                                     att[h][:, c],
                                     start=(h == 0), stop=(h == HPC - 1))
                nc.vector.tensor_copy(ot[:, j, :], po[:])
            dom = d_out[m].rearrange("j p t -> p j t")
            nc.sync.dma_start(dom[:, 0:2, :], ot[:, 0:2, :])
            nc.sync.dma_start(dom[:, 2:4, :], ot[:, 2:4, :])

        if debug:
            dbg_specs = [
                ("dbg_ssqrow_q", ssqrow_q, [1, T], bf16),
                ("dbg_ssqa_q", ssqa_q, [1, T], bf16),
                ("dbg_srow_q", srow_q, [1, T], f32),
                ("dbg_srow_kv", srow_kv, [1, T], f32),
                ("dbg_skvcol", skvcol, [128, NS], f32),
                ("dbg_qn0", qn[0], [128, T], bf16),
                ("dbg_qpe", qpe, [128, T], bf16),
                ("dbg_kn0", kn[0], [128, T], bf16),
                ("dbg_kpe", kpe, [128, T], bf16),
                ("dbg_v0", vna[0], [128, NS, D_V], bf16),
                ("dbg_att0", att[0], [128, T], bf16),
                ("dbg_den", den_dbg, [1, T], f32),
                ("dbg_es00", es_dbg, [128, TCOL], bf16),
                ("dbg_st00", st_dbg, [128, TCOL], f32),
            ]
            for nm, src_t, shp, dt in dbg_specs:
                dd = nc.dram_tensor(nm, shp, dt, kind="ExternalOutput").ap()
                nc.sync.dma_start(dd[:], src_t[:])

    nc.compile()
    return nc


def _host_prep(positions, hidden_states, q_a_w, q_a_ln_w, q_b_w,
               kv_a_w, kv_a_ln_w, kv_b_w, o_w):
    pos = np.asarray(positions, dtype=np.float32)
    hs = np.asarray(hidden_states, dtype=np.float32)
    q_a_w = np.asarray(q_a_w, dtype=np.float32)
    q_b_w = np.asarray(q_b_w, dtype=np.float32) * np.asarray(
        q_a_ln_w, dtype=np.float32)[:, None]
    kv_a_w = np.asarray(kv_a_w, dtype=np.float32)
    kv_b_w = np.asarray(kv_b_w, dtype=np.float32) * np.asarray(
        kv_a_ln_w, dtype=np.float32)[:, None]
    o_w = np.asarray(o_w, dtype=np.float32)

    # fused weights
    wq_full = (q_a_w @ q_b_w).reshape(HIDDEN, H, QK_DIM) * SCALE
    kvb = kv_b_w.reshape(KV_RANK, H, D_NOPE + D_V)
    wk_full = kv_a_w[:, :KV_RANK] @ kvb[:, :, :D_NOPE].reshape(KV_RANK, -1)
    wk_full = wk_full.reshape(HIDDEN, H, D_NOPE)
    wv_full = kv_a_w[:, :KV_RANK] @ kvb[:, :, D_NOPE:].reshape(KV_RANK, -1)
    wv_full = wv_full.reshape(HIDDEN, H, D_V)

    # rope pair permutation: interleaved (0::2, 1::2) -> (x1 block | x2 block)
    qpe_cols = wq_full[:, :, D_NOPE:]
    qpe_perm = np.concatenate([qpe_cols[:, :, 0::2], qpe_cols[:, :, 1::2]],
                              axis=2)  # [HIDDEN, H, 64]
    wkpe = kv_a_w[:, KV_RANK:]
    wkpe_perm = np.concatenate([wkpe[:, 0::2], wkpe[:, 1::2]], axis=1)

    inv_freq = 1.0 / (ROPE_BASE ** (np.arange(0, D_ROPE, 2,
                                              dtype=np.float32) / D_ROPE))
    freqs = pos[None, :] * inv_freq[:, None]           # [32, T]
    cosT = np.tile(np.cos(freqs).astype(np.float32), (4, 1))   # [128, T]
    sinT = np.tile(np.sin(freqs).astype(np.float32), (4, 1))

    # band-swap-with-sign matrix: o = e + Msw @ f  (per 64-row block:
    # rows 0:32 get -f[32:64], rows 32:64 get +f[0:32])
    msw = np.zeros((128, 128), dtype=np.float32)
    for q in range(2):
        for i in range(32):
            msw[64 * q + i, 64 * q + 32 + i] = -1.0
            msw[64 * q + 32 + i, 64 * q + i] = 1.0
    mswT = np.ascontiguousarray(msw.T).astype(BF16)

    # big causal mask: maskb[s, col] = 0 if col >= s + 384 else NEG
    col = np.arange(896)[None, :]
    s_ = np.arange(128)[:, None]
    maskb = np.where(col >= s_ + 384, 0.0, NEG).astype(BF16)

    xt = np.ascontiguousarray(
        hs.T.reshape(NK, 128, NJ, TCOL).transpose(2, 0, 1, 3)).astype(BF16)

    in_maps = []
    for cidx in range(N_CORES):
        h0 = HPC * cidx
        wq_c = np.concatenate(
            [wq_full[:, h0 + h, :D_NOPE] for h in range(HPC)]
            + [qpe_perm[:, h0 + h, :] for h in range(HPC)], axis=1)
        wk_c = np.concatenate(
            [wk_full[:, h0 + h, :] for h in range(HPC)], axis=1)
        wv_c = np.concatenate(
            [wv_full[:, h0 + h, :] for h in range(HPC)], axis=1)
        ws1 = q_a_w[:, 192 * cidx:192 * cidx + 128]
        ws2 = np.concatenate(
            [q_a_w[:, 192 * cidx + 128:192 * (cidx + 1)],
             kv_a_w[:, 64 * cidx:64 * (cidx + 1)]], axis=1)
        ow_c = o_w[D_V * h0:D_V * (h0 + HPC), :]
        in_maps.append({
            "xt": xt,
            "wq": np.ascontiguousarray(wq_c).astype(BF16),
            "wk": np.ascontiguousarray(wk_c).astype(BF16),
            "wv": np.ascontiguousarray(wv_c).astype(BF16),
            "wkpe": np.ascontiguousarray(wkpe_perm).astype(BF16),
            "wssq1": np.ascontiguousarray(ws1).astype(BF16),
            "wssq2": np.ascontiguousarray(ws2).astype(BF16),
            "ow": np.ascontiguousarray(ow_c).astype(BF16),
            "cosT": cosT,
            "sinT": sinT,
            "mswT": mswT,
            "maskbig": maskb,
        })
    return in_maps


def kernel(**inputs):
    from concourse.bass_utils import run_bass_kernel_spmd

    dbg = bool(int(os.environ.get("BASSK_DEBUG", "0")))
    key = "nc_dbg" if dbg else "nc"
    if key not in _CACHE:
        _CACHE[key] = _build_program(debug=dbg)
    nc = _CACHE[key]

    in_maps = _host_prep(**inputs)
    trace = bool(int(os.environ.get("BASSK_TRACE", "0")))
    tmpdir = os.environ.get("BASSK_TMPDIR") or None
    if tmpdir:
        os.makedirs(tmpdir, exist_ok=True)
    res = run_bass_kernel_spmd(nc, in_maps, core_ids=list(range(N_CORES)),
                               trace=trace, tmpdir=tmpdir)
    _CACHE["last_exec_time_ns"] = res.exec_time_ns
    _CACHE["last_results"] = res.results
    outT = np.zeros((NK, NJ, 128, TCOL), dtype=np.float32)
    for r in res.results:
        outT += np.asarray(r["out"], dtype=np.float32)
    outT = outT.transpose(0, 2, 1, 3).reshape(HIDDEN, T)
    return np.ascontiguousarray(outT.T)
